# revision 1
# baseline (speedup 1.0000x reference)
"""Distributed GNN (4-layer GraphConv) Bass kernel for 8 TRN2 NeuronCores.

Self-contained: hosts the graph preprocessing (balanced node->window
placement, per-(chunk,window) uniform-K gather schedule), the Bass/Tile
program (windowed int16 dma_gather + strided DVE segment reduce + PE
transposes/matmuls + ACT bias/relu/sigmoid, AllGather per layer), and the
SPMD orchestration.

kernel(**inputs) takes the FULL unsharded inputs of reference.setup_inputs()
and returns the FULL [100000, 1] float32 output.
"""
import numpy as np
import ml_dtypes

from concourse import bass, bacc, tile, mybir
from concourse.masks import make_identity
from concourse.bass_utils import run_bass_kernel_spmd

N = 100000
E = 1600000
D = 64
NC = 8
NLR = 12500
NLP = 12544          # 98 * 128
V = NC * NLP         # 100352
W = 4
WINP = 2 * NLP       # 25088 rows per gather window (< 32768: int16-safe)
NT = NLP // 128      # 98 tiles
SLOT_BUDGET = 128
T_MAX = 8

F32 = mybir.dt.float32
BF16 = mybir.dt.bfloat16
I16 = mybir.dt.int16
AF = mybir.ActivationFunctionType
ALU = mybir.AluOpType


# ---------------------------------------------------------------- planning
def build_plan(edge_index):
    src = np.asarray(edge_index[0], dtype=np.int64)
    dst = np.asarray(edge_index[1], dtype=np.int64)
    deg_in = np.bincount(dst, minlength=N).astype(np.int64)

    order = np.argsort(src, kind="stable")
    s_sorted = src[order]
    d_sorted = dst[order]
    starts = np.searchsorted(s_sorted, np.arange(N + 1))
    outdeg = starts[1:] - starts[:-1]
    node_order = np.argsort(-outdeg, kind="stable")

    C = np.zeros((N, W), np.int32)
    cap = np.full(W, 2 * NLR, np.int64)
    win_of = np.full(N, -1, np.int8)
    for g in node_order:
        a, b = starts[g], starts[g + 1]
        if a < b:
            ds = d_sorted[a:b]
            scores = C[ds].sum(axis=0).astype(np.float64)
        else:
            scores = np.zeros(W)
        scores[cap <= 0] = 1e18
        w = int(np.argmin(scores))
        win_of[g] = w
        cap[w] -= 1
        if a < b:
            np.add.at(C, (ds, w), 1)

    # refinement (f = c^2), small slack then fixup
    cap_used = np.bincount(win_of, minlength=W)
    capmax = 2 * NLR + 16
    for _ in range(2):
        moved = 0
        for g in range(N):
            a, b = starts[g], starts[g + 1]
            if a == b:
                continue
            ds = d_sorted[a:b]
            Cd = C[ds]
            w0 = win_of[g]
            gain_out = (2 * Cd[:, w0] - 1).sum()
            cost_in = (2 * Cd + 1).sum(axis=0)
            cost_in[w0] = gain_out
            cost_in[cap_used >= capmax] = 2**60
            w1 = int(np.argmin(cost_in))
            if cost_in[w1] < gain_out:
                C[ds, w0] -= 1
                np.add.at(C, (ds, w1), 1)
                win_of[g] = w1
                cap_used[w0] -= 1
                cap_used[w1] += 1
                moved += 1
        if moved == 0:
            break
    for w in range(W):
        while cap_used[w] > 2 * NLR:
            cand = np.where(win_of == w)[0]
            wt = int(np.argmin(cap_used))
            best, bestg = None, None
            for g in cand[:3000]:
                a, b = starts[g], starts[g + 1]
                ds = d_sorted[a:b]
                dc = (2 * C[ds, wt] + 1).sum() - (2 * C[ds, w] - 1).sum()
                if best is None or dc < best:
                    best, bestg = dc, g
            g = bestg
            a, b = starts[g], starts[g + 1]
            ds = d_sorted[a:b]
            C[ds, w] -= 1
            np.add.at(C, (ds, wt), 1)
            win_of[g] = wt
            cap_used[w] -= 1
            cap_used[wt] += 1

    # per-window snake split, sorted by (max window count, degree) desc
    maxc = C.max(axis=1).astype(np.int64)
    sort_key = ((63 - np.minimum(maxc, 63)) * 1024
                + (1023 - np.minimum(deg_in, 1023)))
    gperm = np.empty(N, np.int64)
    orig_of = np.full(V, -1, np.int64)
    for w in range(W):
        nodes_w = np.where(win_of == w)[0]
        order_w = nodes_w[np.argsort(sort_key[nodes_w], kind="stable")]
        for half, core in ((0, 2 * w), (1, 2 * w + 1)):
            sel = order_w[half::2]
            ranks = np.arange(sel.shape[0])
            gperm[sel] = core * NLP + ranks
            orig_of[core * NLP + ranks] = sel

    src_p = gperm[src]
    dst_p = gperm[dst]
    win_s = src_p // WINP
    rel_s = (src_p - win_s * WINP).astype(np.int32)

    cnt = np.bincount(dst_p * W + win_s, minlength=V * W).reshape(NC, NLP, W)
    tile_max = cnt.reshape(NC, NT, 128, W).max(axis=(0, 2))

    chunks = []
    t0 = 0
    while t0 < NT:
        T = 1
        K = tile_max[t0].copy()
        while T < T_MAX and t0 + T < NT:
            K2 = np.maximum(K, tile_max[t0 + T])
            if (T + 1) * int(K2.sum()) > SLOT_BUDGET:
                break
            K = K2
            T += 1
        chunks.append((t0, T, [int(k) for k in K]))
        t0 += T

    ek = dst_p * W + win_s
    eorder = np.argsort(ek, kind="stable")
    ek_s = ek[eorder]
    rel_s_s = rel_s[eorder]
    gstarts = np.searchsorted(ek_s, ek_s)
    kpos = np.arange(E) - gstarts

    core_e = (ek_s // W) // NLP
    rank_e = (ek_s // W) % NLP
    win_e = ek_s % W

    chunk_of_tile = np.empty(NT, np.int32)
    tinc_of_tile = np.empty(NT, np.int32)
    ftot = 0
    call_meta = []
    for ci, (tile0, T, K) in enumerate(chunks):
        chunk_of_tile[tile0:tile0 + T] = ci
        tinc_of_tile[tile0:tile0 + T] = np.arange(T)
        for w in range(W):
            n_idx = 128 * T * K[w]
            call_meta.append(dict(chunk=ci, w=w, tile0=tile0, T=T, K=K[w],
                                  ioff=ftot, n_idx=n_idx))
            ftot += n_idx // 16

    tile_e = rank_e // 128
    p_e = rank_e % 128
    ci_e = chunk_of_tile[tile_e]
    tin_e = tinc_of_tile[tile_e]
    ioff_arr = np.zeros((len(chunks), W), np.int64)
    K_arr = np.zeros((len(chunks), W), np.int64)
    for m in call_meta:
        ioff_arr[m["chunk"], m["w"]] = m["ioff"]
        K_arr[m["chunk"], m["w"]] = m["K"]
    Kk = K_arr[ci_e, win_e]
    j = (tin_e * Kk + kpos) * 128 + p_e
    col = ioff_arr[ci_e, win_e] + j // 16
    row = j % 16

    G16 = np.full((NC, 16, ftot), np.int16(NLR), np.int16)  # pad -> zero row
    G16[core_e, row, col] = rel_s_s.astype(np.int16)
    gidx = np.tile(G16, (1, 8, 1))

    deg_inv = (1.0 / np.maximum(deg_in, 1)).astype(np.float32)
    deg_inv_perm = np.zeros(V, np.float32)
    deg_inv_perm[gperm] = deg_inv
    deg_inv_perm[orig_of < 0] = 1.0

    return dict(gperm=gperm, orig_of=orig_of, chunks=chunks,
                call_meta=call_meta, ftot=ftot, gidx=gidx,
                deg_inv_perm=deg_inv_perm)


# ---------------------------------------------------------------- program
def build_program(plan, n_cores=NC):
    chunks = plan["chunks"]
    call_meta = plan["call_meta"]
    ftot = plan["ftot"]

    nc = bacc.Bacc("TRN2", target_bir_lowering=False, debug=False,
                   num_devices=n_cores, num_swdge_queues=1)

    xtab = nc.dram_tensor("xtab", [V, D], F32, kind="ExternalInput")
    xt = nc.dram_tensor("xt", [D, NLP], BF16, kind="ExternalInput")
    gidx = nc.dram_tensor("gidx", [128, ftot], I16, kind="ExternalInput")
    dgb = nc.dram_tensor("dgb", [128, NT * D], BF16, kind="ExternalInput")
    wts = nc.dram_tensor("wts", [D, 8 * D], BF16, kind="ExternalInput")
    bias = nc.dram_tensor("bias", [D, 4], F32, kind="ExternalInput")
    out = nc.dram_tensor("out", [1, NLP], F32, kind="ExternalOutput")

    with tile.TileContext(nc) as tc:
        with tc.tile_pool(name="const", bufs=1) as constp, \
             tc.tile_pool(name="hT", bufs=1) as hTp, \
             tc.tile_pool(name="idx", bufs=2) as idxp, \
             tc.tile_pool(name="msg", bufs=2) as msgp, \
             tc.tile_pool(name="part", bufs=2) as partp, \
             tc.tile_pool(name="agg", bufs=2) as aggp, \
             tc.tile_pool(name="aggT", bufs=3) as aggTp, \
             tc.tile_pool(name="stage", bufs=2) as stagep, \
             tc.tile_pool(name="psA", bufs=2, space="PSUM") as psA, \
             tc.tile_pool(name="psB", bufs=2, space="PSUM") as psB, \
             tc.tile_pool(name="dram", bufs=1, space="DRAM") as dramp:

            ident = constp.tile([128, 128], BF16)
            make_identity(nc, ident[:])
            identf = constp.tile([128, 128], F32)
            make_identity(nc, identf[:])
            wts_sb = constp.tile([D, 8 * D], BF16)
            nc.sync.dma_start(out=wts_sb[:], in_=wts.ap()[:])
            bias_sb = constp.tile([D, 4], F32)
            nc.sync.dma_start(out=bias_sb[:], in_=bias.ap()[:])
            dgb_sb = constp.tile([128, NT * D], BF16)
            nc.sync.dma_start(out=dgb_sb[:], in_=dgb.ap()[:])

            hT_a = hTp.tile([D, NLP], BF16, name="hT_a")
            hT_b = hTp.tile([D, NLP], BF16, name="hT_b")
            nc.sync.dma_start(out=hT_a[:], in_=xt.ap()[:])

            agin = dramp.tile([NLP, D], F32, name="agin")
            tabsr = [dramp.tile([V, D], F32, name=f"tabr{i}",
                                addr_space="Shared") for i in range(3)]

            ci_meta = {}
            for m in call_meta:
                ci_meta.setdefault(m["chunk"], []).append(m)

            def layer(l, hT_in, hT_out):
                tbl = xtab.ap() if l == 0 else tabsr[l - 1][:]
                last = l == 3
                wself = wts_sb[:, (2 * l) * D:
                               (2 * l) * D + (1 if last else D)]
                wneigh = wts_sb[:, (2 * l + 1) * D:
                                (2 * l + 1) * D + (1 if last else D)]
                bias_ap = bias_sb[0:1 if last else D, l:l + 1]
                MOUT = 1 if last else D

                for ci, (tile0, T, K) in enumerate(chunks):
                    SK = sum(K)
                    ms = ci_meta[ci]
                    i0 = ms[0]["ioff"]
                    i1 = ms[-1]["ioff"] + ms[-1]["n_idx"] // 16
                    idx_t = idxp.tile([128, i1 - i0], I16, tag="idx")
                    nc.sync.dma_start(out=idx_t[:], in_=gidx.ap()[:, i0:i1])

                    msg = msgp.tile([128, T * SK * D], F32, tag="msg")
                    part = partp.tile([128, W * T * D], F32, tag="part")
                    off = 0
                    for m in ms:
                        w, Kw, n_idx = m["w"], m["K"], m["n_idx"]
                        if Kw == 0:
                            nc.vector.memset(
                                part[:, w * T * D:(w + 1) * T * D], 0.0)
                            continue
                        nc.gpsimd.dma_gather(
                            msg[:, off * D:(off + T * Kw) * D].rearrange(
                                "p (s e) -> p s e", s=T * Kw, e=D),
                            tbl[w * WINP:(w + 1) * WINP, :],
                            idx_t[:, m["ioff"] - i0:
                                  m["ioff"] - i0 + n_idx // 16],
                            n_idx, n_idx, D, elem_step=D, queue_num=0,
                            single_packet=False,
                        )
                        nc.vector.tensor_reduce(
                            out=part[:, w * T * D:(w + 1) * T * D].rearrange(
                                "p (t e) -> p t e", t=T, e=D),
                            in_=msg[:, off * D:(off + T * Kw) * D].rearrange(
                                "p (t k e) -> p t e k", t=T, k=Kw, e=D),
                            axis=mybir.AxisListType.X, op=ALU.add,
                        )
                        off += T * Kw

                    agg = aggp.tile([128, T * D], F32, tag="agg")
                    nc.vector.tensor_reduce(
                        out=agg[:], in_=part[:].rearrange(
                            "p (w s) -> p s w", w=W, s=T * D),
                        axis=mybir.AxisListType.X, op=ALU.add,
                    )
                    nc.vector.tensor_tensor(
                        out=agg[:], in0=agg[:],
                        in1=dgb_sb[:, tile0 * D:(tile0 + T) * D],
                        op=ALU.mult,
                    )

                    bt = 0
                    while bt < T:
                        Tb = min(4, T - bt)
                        cols = slice((tile0 + bt) * 128,
                                     (tile0 + bt + Tb) * 128)
                        aggT_ps = psA.tile([D, Tb * 128], F32, tag="aggT_ps")
                        for tt in range(Tb):
                            nc.tensor.transpose(
                                out=aggT_ps[:, tt * 128:(tt + 1) * 128],
                                in_=agg[:, (bt + tt) * D:(bt + tt + 1) * D],
                                identity=identf[:],
                            )
                        aggT_sb = aggTp.tile([D, Tb * 128], BF16, tag="aggT")
                        nc.vector.tensor_copy(out=aggT_sb[:], in_=aggT_ps[:])

                        out_ps = psB.tile([MOUT, Tb * 128], F32, tag="out_ps")
                        nc.tensor.matmul(out=out_ps[:], lhsT=wneigh,
                                         rhs=aggT_sb[:], start=True,
                                         stop=False)
                        nc.tensor.matmul(out=out_ps[:], lhsT=wself,
                                         rhs=hT_in[:, cols], start=False,
                                         stop=True)
                        if last:
                            osb = aggTp.tile([1, 512], F32, tag="osb")
                            nc.scalar.activation(
                                out=osb[0:1, :Tb * 128], in_=out_ps[:],
                                func=AF.Sigmoid, bias=bias_ap)
                            nc.sync.dma_start(out=out.ap()[0:1, cols],
                                              in_=osb[0:1, :Tb * 128])
                        else:
                            nc.scalar.activation(
                                out=hT_out[:, cols], in_=out_ps[:],
                                func=AF.Relu, bias=bias_ap)
                        bt += Tb

                if last:
                    return
                nc.vector.memset(hT_out[:, NLR:NLP], 0.0)
                g0 = 0
                while g0 < NT:
                    Tg = min(8, NT - g0)
                    st_ps = psA.tile([128, Tg * D], BF16, tag="st_ps")
                    for t in range(Tg):
                        nc.tensor.transpose(
                            out=st_ps[:, t * D:(t + 1) * D],
                            in_=hT_out[:, (g0 + t) * 128:(g0 + t + 1) * 128],
                            identity=ident[:D, :D],
                        )
                    st_sb = stagep.tile([128, Tg * D], F32, tag="st")
                    nc.vector.tensor_copy(out=st_sb[:], in_=st_ps[:])
                    nc.sync.dma_start(
                        out=agin[g0 * 128:(g0 + Tg) * 128, :].rearrange(
                            "(t p) f -> p t f", t=Tg, p=128),
                        in_=st_sb[:].rearrange("p (t f) -> p t f",
                                               t=Tg, f=D),
                    )
                    g0 += Tg
                nc.gpsimd.collective_compute(
                    "AllGather", ALU.bypass,
                    replica_groups=[list(range(n_cores))],
                    ins=[agin.opt()], outs=[tabsr[l].opt()],
                )

            layer(0, hT_a, hT_b)
            layer(1, hT_b, hT_a)
            layer(2, hT_a, hT_b)
            layer(3, hT_b, None)

    nc.compile()
    return nc


# ------------------------------------------------------------------ driver
def kernel(x, edge_index, Wself1, Wneigh1, b1, Wself2, Wneigh2, b2,
           Wself3, Wneigh3, b3, Wself4, Wneigh4, b4):
    x = np.asarray(x, dtype=np.float32)
    edge_index = np.asarray(edge_index)
    plan = build_plan(edge_index)
    gperm = plan["gperm"]

    xtab = np.zeros((V, D), np.float32)
    xtab[gperm] = x
    deginv = plan["deg_inv_perm"]

    wts = np.zeros((D, 8 * D), np.float32)
    for l, (ws, wn) in enumerate(((Wself1, Wneigh1), (Wself2, Wneigh2),
                                  (Wself3, Wneigh3), (Wself4, Wneigh4))):
        ws = np.asarray(ws, np.float32)
        wn = np.asarray(wn, np.float32)
        wts[:, 2 * l * D:2 * l * D + ws.shape[1]] = ws
        wts[:, (2 * l + 1) * D:(2 * l + 1) * D + wn.shape[1]] = wn
    wts = wts.astype(ml_dtypes.bfloat16)
    bias = np.zeros((D, 4), np.float32)
    for l, b in enumerate((b1, b2, b3, b4)):
        b = np.asarray(b, np.float32)
        bias[:, l] = b[0] if b.shape[0] == 1 else b

    in_maps = []
    for c in range(NC):
        sl = slice(c * NLP, (c + 1) * NLP)
        xt = np.ascontiguousarray(xtab[sl].T).astype(ml_dtypes.bfloat16)
        dgb = np.broadcast_to(deginv[sl].reshape(NT, 128, 1), (NT, 128, D))
        dgb = np.ascontiguousarray(
            dgb.transpose(1, 0, 2).reshape(128, NT * D)
        ).astype(ml_dtypes.bfloat16)
        in_maps.append(dict(xtab=xtab, xt=xt, gidx=plan["gidx"][c],
                            dgb=dgb, wts=wts, bias=bias))

    nc = build_program(plan, n_cores=NC)
    res = run_bass_kernel_spmd(nc, in_maps, core_ids=list(range(NC)))

    out_perm = np.concatenate(
        [np.asarray(res.results[c]["out"]).reshape(-1)[:NLR]
         for c in range(NC)])
    orig = np.concatenate([plan["orig_of"][c * NLP:c * NLP + NLR]
                           for c in range(NC)])
    out_full = np.empty(N, np.float32)
    out_full[orig] = out_perm
    return out_full.reshape(N, 1)



# revision 6
# speedup vs baseline: 17.8964x; 17.8964x over previous
"""Distributed GNN (4-layer GraphConv) Bass kernel for 8 TRN2 NeuronCores.

Self-contained: hosts the graph preprocessing (balanced node->window
placement, per-(chunk,window) uniform-K gather schedule), the Bass/Tile
program (windowed int16 dma_gather + strided DVE segment reduce + PE
transposes/matmuls + ACT bias/relu/sigmoid, AllGather per layer), and the
SPMD orchestration.

kernel(**inputs) takes the FULL unsharded inputs of reference.setup_inputs()
and returns the FULL [100000, 1] float32 output.
"""
import numpy as np
import ml_dtypes

from concourse import bass, bacc, tile, mybir
from concourse.masks import make_identity
from concourse.bass_utils import run_bass_kernel_spmd

N = 100000
E = 1600000
D = 64
NC = 8
NLR = 12500
NLP = 12544          # 98 * 128
V = NC * NLP         # 100352
W = 4
WINP = 2 * NLP       # 25088 rows per gather window (< 32768: int16-safe)
NT = NLP // 128      # 98 tiles
SLOT_BUDGET = 128
T_MAX = 8

F32 = mybir.dt.float32
BF16 = mybir.dt.bfloat16
I16 = mybir.dt.int16
AF = mybir.ActivationFunctionType
ALU = mybir.AluOpType


# ---------------------------------------------------------------- planning
def build_plan(edge_index):
    src = np.asarray(edge_index[0], dtype=np.int64)
    dst = np.asarray(edge_index[1], dtype=np.int64)
    deg_in = np.bincount(dst, minlength=N).astype(np.int64)

    order = np.argsort(src, kind="stable")
    s_sorted = src[order]
    d_sorted = dst[order]
    starts = np.searchsorted(s_sorted, np.arange(N + 1))
    outdeg = starts[1:] - starts[:-1]
    node_order = np.argsort(-outdeg, kind="stable")

    C = np.zeros((N, W), np.int32)
    cap = np.full(W, 2 * NLR, np.int64)
    win_of = np.full(N, -1, np.int8)
    for g in node_order:
        a, b = starts[g], starts[g + 1]
        if a < b:
            ds = d_sorted[a:b]
            scores = C[ds].sum(axis=0).astype(np.float64)
        else:
            scores = np.zeros(W)
        scores[cap <= 0] = 1e18
        w = int(np.argmin(scores))
        win_of[g] = w
        cap[w] -= 1
        if a < b:
            np.add.at(C, (ds, w), 1)

    # refinement (f = c^2), small slack then fixup
    cap_used = np.bincount(win_of, minlength=W)
    capmax = 2 * NLR + 16
    for _ in range(2):
        moved = 0
        for g in range(N):
            a, b = starts[g], starts[g + 1]
            if a == b:
                continue
            ds = d_sorted[a:b]
            Cd = C[ds]
            w0 = win_of[g]
            gain_out = (2 * Cd[:, w0] - 1).sum()
            cost_in = (2 * Cd + 1).sum(axis=0)
            cost_in[w0] = gain_out
            cost_in[cap_used >= capmax] = 2**60
            w1 = int(np.argmin(cost_in))
            if cost_in[w1] < gain_out:
                C[ds, w0] -= 1
                np.add.at(C, (ds, w1), 1)
                win_of[g] = w1
                cap_used[w0] -= 1
                cap_used[w1] += 1
                moved += 1
        if moved == 0:
            break
    for w in range(W):
        while cap_used[w] > 2 * NLR:
            cand = np.where(win_of == w)[0]
            wt = int(np.argmin(cap_used))
            best, bestg = None, None
            for g in cand[:3000]:
                a, b = starts[g], starts[g + 1]
                ds = d_sorted[a:b]
                dc = (2 * C[ds, wt] + 1).sum() - (2 * C[ds, w] - 1).sum()
                if best is None or dc < best:
                    best, bestg = dc, g
            g = bestg
            a, b = starts[g], starts[g + 1]
            ds = d_sorted[a:b]
            C[ds, w] -= 1
            np.add.at(C, (ds, wt), 1)
            win_of[g] = wt
            cap_used[w] -= 1
            cap_used[wt] += 1

    # per-window snake split, sorted by (max window count, degree) desc
    maxc = C.max(axis=1).astype(np.int64)
    sort_key = ((63 - np.minimum(maxc, 63)) * 1024
                + (1023 - np.minimum(deg_in, 1023)))
    gperm = np.empty(N, np.int64)
    orig_of = np.full(V, -1, np.int64)
    for w in range(W):
        nodes_w = np.where(win_of == w)[0]
        order_w = nodes_w[np.argsort(sort_key[nodes_w], kind="stable")]
        for half, core in ((0, 2 * w), (1, 2 * w + 1)):
            sel = order_w[half::2]
            ranks = np.arange(sel.shape[0])
            gperm[sel] = core * NLP + ranks
            orig_of[core * NLP + ranks] = sel

    src_p = gperm[src]
    dst_p = gperm[dst]
    win_s = src_p // WINP
    rel_s = (src_p - win_s * WINP).astype(np.int32)

    cnt = np.bincount(dst_p * W + win_s, minlength=V * W).reshape(NC, NLP, W)
    tile_max = cnt.reshape(NC, NT, 128, W).max(axis=(0, 2))

    chunks = []
    t0 = 0
    while t0 < NT:
        T = 1
        K = tile_max[t0].copy()
        while T < T_MAX and t0 + T < NT:
            K2 = np.maximum(K, tile_max[t0 + T])
            if (T + 1) * int(K2.sum()) > SLOT_BUDGET:
                break
            K = K2
            T += 1
        chunks.append((t0, T, [int(k) for k in K]))
        t0 += T

    ek = dst_p * W + win_s
    eorder = np.argsort(ek, kind="stable")
    ek_s = ek[eorder]
    rel_s_s = rel_s[eorder]
    gstarts = np.searchsorted(ek_s, ek_s)
    kpos = np.arange(E) - gstarts

    core_e = (ek_s // W) // NLP
    rank_e = (ek_s // W) % NLP
    win_e = ek_s % W

    chunk_of_tile = np.empty(NT, np.int32)
    tinc_of_tile = np.empty(NT, np.int32)
    ftot = 0
    call_meta = []
    for ci, (tile0, T, K) in enumerate(chunks):
        chunk_of_tile[tile0:tile0 + T] = ci
        tinc_of_tile[tile0:tile0 + T] = np.arange(T)
        for w in range(W):
            n_idx = 128 * T * K[w]
            call_meta.append(dict(chunk=ci, w=w, tile0=tile0, T=T, K=K[w],
                                  ioff=ftot, n_idx=n_idx))
            ftot += n_idx // 16

    tile_e = rank_e // 128
    p_e = rank_e % 128
    ci_e = chunk_of_tile[tile_e]
    tin_e = tinc_of_tile[tile_e]
    ioff_arr = np.zeros((len(chunks), W), np.int64)
    K_arr = np.zeros((len(chunks), W), np.int64)
    for m in call_meta:
        ioff_arr[m["chunk"], m["w"]] = m["ioff"]
        K_arr[m["chunk"], m["w"]] = m["K"]
    Kk = K_arr[ci_e, win_e]
    j = (tin_e * Kk + kpos) * 128 + p_e
    col = ioff_arr[ci_e, win_e] + j // 16
    row = j % 16

    G16 = np.full((NC, 16, ftot), np.int16(NLR), np.int16)  # pad -> zero row
    G16[core_e, row, col] = rel_s_s.astype(np.int16)
    gidx = np.tile(G16, (1, 8, 1))

    deg_inv = (1.0 / np.maximum(deg_in, 1)).astype(np.float32)
    deg_inv_perm = np.zeros(V, np.float32)
    deg_inv_perm[gperm] = deg_inv
    deg_inv_perm[orig_of < 0] = 1.0

    return dict(gperm=gperm, orig_of=orig_of, chunks=chunks,
                call_meta=call_meta, ftot=ftot, gidx=gidx,
                deg_inv_perm=deg_inv_perm)


# ---------------------------------------------------------------- program
def build_program(plan, n_cores=NC):
    chunks = plan["chunks"]
    call_meta = plan["call_meta"]
    ftot = plan["ftot"]

    nc = bacc.Bacc("TRN2", target_bir_lowering=False, debug=False,
                   num_devices=n_cores, num_swdge_queues=4)

    xtab = nc.dram_tensor("xtab", [V, D], F32, kind="ExternalInput")
    xt = nc.dram_tensor("xt", [D, NLP], BF16, kind="ExternalInput")
    gidx = nc.dram_tensor("gidx", [128, ftot], I16, kind="ExternalInput")
    dgb = nc.dram_tensor("dgb", [128, NT * D], BF16, kind="ExternalInput")
    wts = nc.dram_tensor("wts", [D, 8 * D], BF16, kind="ExternalInput")
    bias = nc.dram_tensor("bias", [D, 4], F32, kind="ExternalInput")
    out = nc.dram_tensor("out", [1, NLP], F32, kind="ExternalOutput")

    with tile.TileContext(nc) as tc:
        with tc.tile_pool(name="const", bufs=1) as constp, \
             tc.tile_pool(name="hT", bufs=1) as hTp, \
             tc.tile_pool(name="idx", bufs=3) as idxp, \
             tc.tile_pool(name="msg", bufs=3) as msgp, \
             tc.tile_pool(name="part", bufs=3) as partp, \
             tc.tile_pool(name="agg", bufs=2) as aggp, \
             tc.tile_pool(name="aggT", bufs=3) as aggTp, \
             tc.tile_pool(name="stage", bufs=2) as stagep, \
             tc.tile_pool(name="psA", bufs=2, space="PSUM") as psA, \
             tc.tile_pool(name="psB", bufs=2, space="PSUM") as psB, \
             tc.tile_pool(name="dram", bufs=1, space="DRAM") as dramp:

            ident = constp.tile([128, 128], BF16)
            make_identity(nc, ident[:])
            identf = constp.tile([128, 128], F32)
            make_identity(nc, identf[:])
            wts_sb = constp.tile([D, 8 * D], BF16)
            nc.sync.dma_start(out=wts_sb[:], in_=wts.ap()[:])
            bias_sb = constp.tile([D, 4], F32)
            nc.sync.dma_start(out=bias_sb[:], in_=bias.ap()[:])
            dgb_sb = constp.tile([128, NT * D], BF16)
            nc.sync.dma_start(out=dgb_sb[:], in_=dgb.ap()[:])

            hT_a = hTp.tile([D, NLP], BF16, name="hT_a")
            hT_b = hTp.tile([D, NLP], BF16, name="hT_b")
            nc.sync.dma_start(out=hT_a[:], in_=xt.ap()[:])

            agin = dramp.tile([NLP, D], F32, name="agin")
            tabsr = [dramp.tile([V, D], F32, name=f"tabr{i}",
                                addr_space="Shared") for i in range(3)]

            ci_meta = {}
            for m in call_meta:
                ci_meta.setdefault(m["chunk"], []).append(m)

            # static queue assignment: greedy balance of gather descriptor
            # counts across the 4 SWDGE queues
            qload = [0] * 4
            qassign = {}
            for m in call_meta:
                if m["K"] == 0:
                    continue
                q = min(range(4), key=lambda i: qload[i])
                qassign[(m["chunk"], m["w"])] = q
                qload[q] += m["n_idx"]

            def layer(l, hT_in, hT_out):
                tbl = xtab.ap() if l == 0 else tabsr[l - 1][:]
                last = l == 3
                wself = wts_sb[:, (2 * l) * D:
                               (2 * l) * D + (1 if last else D)]
                wneigh = wts_sb[:, (2 * l + 1) * D:
                                (2 * l + 1) * D + (1 if last else D)]
                bias_ap = bias_sb[0:1 if last else D, l:l + 1]
                MOUT = 1 if last else D

                for ci, (tile0, T, K) in enumerate(chunks):
                    SK = sum(K)
                    ms = ci_meta[ci]
                    i0 = ms[0]["ioff"]
                    i1 = ms[-1]["ioff"] + ms[-1]["n_idx"] // 16
                    idx_t = idxp.tile([128, i1 - i0], I16, tag="idx")
                    nc.sync.dma_start(out=idx_t[:], in_=gidx.ap()[:, i0:i1])

                    msg = msgp.tile([128, T * SK * D], F32, tag="msg")
                    part = partp.tile([128, W * T * D], F32, tag="part")
                    off = 0
                    for m in ms:
                        w, Kw, n_idx = m["w"], m["K"], m["n_idx"]
                        if Kw == 0:
                            nc.vector.memset(
                                part[:, w * T * D:(w + 1) * T * D], 0.0)
                            continue
                        nc.gpsimd.dma_gather(
                            msg[:, off * D:(off + T * Kw) * D].rearrange(
                                "p (s e) -> p s e", s=T * Kw, e=D),
                            tbl[w * WINP:(w + 1) * WINP, :],
                            idx_t[:, m["ioff"] - i0:
                                  m["ioff"] - i0 + n_idx // 16],
                            n_idx, n_idx, D, elem_step=D,
                            queue_num=qassign[(ci, w)],
                            single_packet=False,
                        )
                        nc.vector.tensor_reduce(
                            out=part[:, w * T * D:(w + 1) * T * D].rearrange(
                                "p (t e) -> p t e", t=T, e=D),
                            in_=msg[:, off * D:(off + T * Kw) * D].rearrange(
                                "p (t k e) -> p t e k", t=T, k=Kw, e=D),
                            axis=mybir.AxisListType.X, op=ALU.add,
                        )
                        off += T * Kw

                    agg = aggp.tile([128, T * D], F32, tag="agg")
                    nc.vector.tensor_reduce(
                        out=agg[:], in_=part[:].rearrange(
                            "p (w s) -> p s w", w=W, s=T * D),
                        axis=mybir.AxisListType.X, op=ALU.add,
                    )
                    nc.vector.tensor_tensor(
                        out=agg[:], in0=agg[:],
                        in1=dgb_sb[:, tile0 * D:(tile0 + T) * D],
                        op=ALU.mult,
                    )

                    bt = 0
                    while bt < T:
                        Tb = min(4, T - bt)
                        cols = slice((tile0 + bt) * 128,
                                     (tile0 + bt + Tb) * 128)
                        aggT_ps = psA.tile([D, Tb * 128], F32, tag="aggT_ps")
                        for tt in range(Tb):
                            nc.tensor.transpose(
                                out=aggT_ps[:, tt * 128:(tt + 1) * 128],
                                in_=agg[:, (bt + tt) * D:(bt + tt + 1) * D],
                                identity=identf[:],
                            )
                        aggT_sb = aggTp.tile([D, Tb * 128], BF16, tag="aggT")
                        nc.vector.tensor_copy(out=aggT_sb[:], in_=aggT_ps[:])

                        out_ps = psB.tile([MOUT, Tb * 128], F32, tag="out_ps")
                        nc.tensor.matmul(out=out_ps[:], lhsT=wneigh,
                                         rhs=aggT_sb[:], start=True,
                                         stop=False)
                        nc.tensor.matmul(out=out_ps[:], lhsT=wself,
                                         rhs=hT_in[:, cols], start=False,
                                         stop=True)
                        if last:
                            osb = aggTp.tile([1, 512], F32, tag="osb")
                            nc.scalar.activation(
                                out=osb[0:1, :Tb * 128], in_=out_ps[:],
                                func=AF.Sigmoid, bias=bias_ap)
                            nc.sync.dma_start(out=out.ap()[0:1, cols],
                                              in_=osb[0:1, :Tb * 128])
                        else:
                            nc.scalar.activation(
                                out=hT_out[:, cols], in_=out_ps[:],
                                func=AF.Relu, bias=bias_ap)
                        bt += Tb

                if last:
                    return
                nc.vector.memset(hT_out[:, NLR:NLP], 0.0)
                g0 = 0
                while g0 < NT:
                    Tg = min(8, NT - g0)
                    st_ps = psA.tile([128, Tg * D], BF16, tag="st_ps")
                    for t in range(Tg):
                        nc.tensor.transpose(
                            out=st_ps[:, t * D:(t + 1) * D],
                            in_=hT_out[:, (g0 + t) * 128:(g0 + t + 1) * 128],
                            identity=ident[:D, :D],
                        )
                    st_sb = stagep.tile([128, Tg * D], F32, tag="st")
                    nc.vector.tensor_copy(out=st_sb[:], in_=st_ps[:])
                    nc.sync.dma_start(
                        out=agin[g0 * 128:(g0 + Tg) * 128, :].rearrange(
                            "(t p) f -> p t f", t=Tg, p=128),
                        in_=st_sb[:].rearrange("p (t f) -> p t f",
                                               t=Tg, f=D),
                    )
                    g0 += Tg
                nc.gpsimd.collective_compute(
                    "AllGather", ALU.bypass,
                    replica_groups=[list(range(n_cores))],
                    ins=[agin.opt()], outs=[tabsr[l].opt()],
                )

            layer(0, hT_a, hT_b)
            layer(1, hT_b, hT_a)
            layer(2, hT_a, hT_b)
            layer(3, hT_b, None)

    nc.compile()
    return nc


# ------------------------------------------------------------------ driver
def build_in_maps(inputs, plan):
    x = np.asarray(inputs["x"], dtype=np.float32)
    gperm = plan["gperm"]

    xtab = np.zeros((V, D), np.float32)
    xtab[gperm] = x
    deginv = plan["deg_inv_perm"]

    wts = np.zeros((D, 8 * D), np.float32)
    for l in range(4):
        ws = np.asarray(inputs[f"Wself{l + 1}"], np.float32)
        wn = np.asarray(inputs[f"Wneigh{l + 1}"], np.float32)
        wts[:, 2 * l * D:2 * l * D + ws.shape[1]] = ws
        wts[:, (2 * l + 1) * D:(2 * l + 1) * D + wn.shape[1]] = wn
    wts = wts.astype(ml_dtypes.bfloat16)
    bias = np.zeros((D, 4), np.float32)
    for l in range(4):
        b = np.asarray(inputs[f"b{l + 1}"], np.float32)
        bias[:, l] = b[0] if b.shape[0] == 1 else b

    in_maps = []
    for c in range(NC):
        sl = slice(c * NLP, (c + 1) * NLP)
        xt = np.ascontiguousarray(xtab[sl].T).astype(ml_dtypes.bfloat16)
        dgb = np.broadcast_to(deginv[sl].reshape(NT, 128, 1), (NT, 128, D))
        dgb = np.ascontiguousarray(
            dgb.transpose(1, 0, 2).reshape(128, NT * D)
        ).astype(ml_dtypes.bfloat16)
        in_maps.append(dict(xtab=xtab, xt=xt, gidx=plan["gidx"][c],
                            dgb=dgb, wts=wts, bias=bias))
    return in_maps


def kernel(x, edge_index, Wself1, Wneigh1, b1, Wself2, Wneigh2, b2,
           Wself3, Wneigh3, b3, Wself4, Wneigh4, b4):
    edge_index = np.asarray(edge_index)
    plan = build_plan(edge_index)
    inputs = dict(x=x, Wself1=Wself1, Wneigh1=Wneigh1, b1=b1,
                  Wself2=Wself2, Wneigh2=Wneigh2, b2=b2,
                  Wself3=Wself3, Wneigh3=Wneigh3, b3=b3,
                  Wself4=Wself4, Wneigh4=Wneigh4, b4=b4)
    in_maps = build_in_maps(inputs, plan)

    nc = build_program(plan, n_cores=NC)
    res = run_bass_kernel_spmd(nc, in_maps, core_ids=list(range(NC)))

    out_perm = np.concatenate(
        [np.asarray(res.results[c]["out"]).reshape(-1)[:NLR]
         for c in range(NC)])
    orig = np.concatenate([plan["orig_of"][c * NLP:c * NLP + NLR]
                           for c in range(NC)])
    out_full = np.empty(N, np.float32)
    out_full[orig] = out_perm
    return out_full.reshape(N, 1)

